# revision 2
# baseline (speedup 1.0000x reference)
"""Multi-head attention on 8 trn2 NeuronCores — v2.

Strategy vs the v1 kernel:
- Masked-key compaction: the mask is per-key (shape [B,1,1,S]) and known on
  the host, so masked keys are dropped before the K/V projections and
  attention.  ~51% of keys are masked for this input distribution, halving
  score/attn/exp/K-V-projection work.  Padding keys (to a 128 multiple)
  carry keep=0 and zeroed V so they drop out of both softmax numerator and
  denominator; no masking is needed anywhere on device.
- Head-sharded (tensor-parallel) Q/K projections: every core projects ALL
  tokens into its own 128 features (2 heads), which lands directly in the
  head-sharded attention layout — no collective between projection and
  attention.  Cost is identical to row-sharding (same output elements), only
  input DMA is bigger (bf16 host-converted).
- V is projected row-sharded (natural token-major layout falls out of the
  matmul) and redistributed with one AllToAll that overlaps the K/Q
  projections and early score tiles.
- All matmuls bf16 (fp8 was measured to cost 3e-2..6e-2 max-rel error —
  over the 2e-2 budget; bf16 end-to-end sits at ~4e-3).  Projections use
  full-PSUM weight-stationary ordering (one 2KB bank per accumulation
  group).  exp folds the 1/sqrt(dk) descale into ScalarE's activation
  scale; softmax denominator comes from a keep-flag column appended to V.
- One AllToAll moves attention output to row sharding for the out
  projection.
"""
import numpy as np

from concourse import bacc, tile, mybir
from concourse.bass_utils import run_bass_kernel_spmd

N_CORES = 8
B, S, D, H = 2, 2048, 1024, 16
DK = D // H                  # 64
R = B * S                    # 4096 query rows
RPC = R // N_CORES           # 512 rows/core for out-projection
HPC = H // N_CORES           # 2 heads/core
KC = D // 128                # 8 contraction chunks
ESC = 1.0 / np.sqrt(DK)      # exp() input descale

dt = mybir.dt
AF = mybir.ActivationFunctionType

_CACHE = {}
OUT_NAMES = ["outT"]


def _build(kt0=8, kt1=8, reps=1, no_collective=False, variant="", PEXP=2):
    KT = kt0 + kt1               # key tiles total; divisible by 8 so V
    assert KT % 8 == 0           # token rows shard evenly across cores
    TKV = 128 * KT               # compacted+padded kv tokens
    KVL = TKV // N_CORES         # local kv token rows (V projection)
    MVT = KVL // 128             # local V row tiles
    MKT = KT // N_CORES          # key tiles delivered per source core

    nc = bacc.Bacc("TRN2", target_bir_lowering=False, debug=False,
                   num_devices=N_CORES)

    f32r = dt.float32r
    bf16 = dt.bfloat16
    rg = [list(range(N_CORES))]

    # ---- per-core DRAM I/O ----
    xqT = nc.dram_tensor("xqT", [D, R], bf16, kind="ExternalInput")
    xkT = nc.dram_tensor("xkT", [D, TKV], bf16, kind="ExternalInput")
    xvT = nc.dram_tensor("xvT", [D, KVL], bf16, kind="ExternalInput")
    wq = nc.dram_tensor("wq", [D, 128], bf16, kind="ExternalInput")
    wk = nc.dram_tensor("wk", [D, 128], bf16, kind="ExternalInput")
    wv = nc.dram_tensor("wv", [D, D], bf16, kind="ExternalInput")
    wo = nc.dram_tensor("wo", [D, D], bf16, kind="ExternalInput")
    bq = nc.dram_tensor("bq", [128], dt.float32, kind="ExternalInput")
    bk = nc.dram_tensor("bk", [128], dt.float32, kind="ExternalInput")
    bv = nc.dram_tensor("bv", [D], dt.float32, kind="ExternalInput")
    bo = nc.dram_tensor("bo", [D], dt.float32, kind="ExternalInput")
    keepin = nc.dram_tensor("keepin", [128, KT], bf16, kind="ExternalInput")
    mv01 = nc.dram_tensor("mv01", [128, MVT], dt.float32,
                          kind="ExternalInput")
    onesin = nc.dram_tensor("onesin", [128, 128], dt.float32,
                            kind="ExternalInput")
    outT = nc.dram_tensor("outT", [D, RPC], dt.float32, kind="ExternalOutput")

    with tile.TileContext(nc) as tc:
        with tc.tile_pool(name="dram", bufs=1, space="DRAM") as dram:
            for rep in range(reps):
                a2v_in = dram.tile([N_CORES, KVL, 128], bf16)
                a2v_out = dram.tile([N_CORES, KVL, 128], bf16)
                a2_in = [dram.tile([N_CORES, 128, RPC // 2], bf16,
                                   name=f"a2in{s}") for s in range(2)]
                a2_out = [dram.tile([N_CORES, 128, RPC // 2], bf16,
                                    name=f"a2out{s}") for s in range(2)]
                qdma = [nc.sync, nc.scalar, nc.gpsimd, nc.sync]

                # ============ phase 1: projections (no collectives) =======
                with (
                    tc.tile_pool(name="p1o", bufs=1) as p1o,
                    tc.tile_pool(name="p1b", bufs=1) as p1b,
                ):
                    p1x = tc.alloc_tile_pool(name="p1x", bufs=1)
                    p1w = tc.alloc_tile_pool(name="p1w", bufs=1)
                    p1psv = tc.alloc_tile_pool(name="p1psv", bufs=2,
                                               space="PSUM")
                    p1ps = tc.alloc_tile_pool(name="p1ps", bufs=1,
                                              space="PSUM")

                    # --- bias / const loads (tiny) ---
                    bq_sb = p1b.tile([128, 1], dt.float32, tag="bq")
                    bk_sb = p1b.tile([128, 1], dt.float32, tag="bk")
                    mv_sb = p1b.tile([128, MVT], dt.float32, tag="mv")
                    if rep == 0:
                        nc.scalar.dma_start(
                            bq_sb[:], bq[:].rearrange("(p one) -> p one", one=1))
                        nc.scalar.dma_start(
                            bk_sb[:], bk[:].rearrange("(p one) -> p one", one=1))
                        nc.scalar.dma_start(mv_sb[:], mv01[:])
                    else:
                        # serialize reps for the x1/x5 timing differencing:
                        # garbage-but-finite values sourced from rep-1's output
                        nc.scalar.dma_start(bq_sb[:], outT[0:128, 0:1])
                        nc.scalar.dma_start(bk_sb[:], outT[0:128, 1:2])
                        nc.scalar.dma_start(mv_sb[:], outT[0:128, 2:2 + MVT])
                    bv_sb = p1b.tile([1, D], f32r, tag="bv")
                    nc.scalar.dma_start(
                        bv_sb[:],
                        bv[:].rearrange("(one f) -> one f", one=1).bitcast(f32r))
                    ones128 = p1b.tile([1, 128], f32r, tag="ones128")
                    nc.scalar.dma_start(ones128[:], onesin[0:1, :].bitcast(f32r))
                    ones64 = p1b.tile([1, 64], f32r, tag="ones64")
                    nc.scalar.dma_start(ones64[:], onesin[0:1, 0:64].bitcast(f32r))
                    bo_sb = p1b.tile([128, KC], dt.float32, tag="bo")
                    nc.scalar.dma_start(
                        bo_sb[:], bo[:].rearrange("(n p) -> p n", p=128))

                    # --- V projection (row-sharded, natural layout) ---
                    xv_sb = p1x.tile([128, KC, KVL], bf16, tag="xv")
                    wv_sb = p1w.tile([128, KC, D], bf16, tag="wv")
                    for t in range(KC):
                        qdma[t % 2].dma_start(
                            xv_sb[:, t], xvT[t * 128:(t + 1) * 128, :])
                        qdma[2 + t % 2].dma_start(
                            wv_sb[:, t], wv[t * 128:(t + 1) * 128, :])
                    v_nat = p1o.tile([128, MVT, D], bf16, tag="vnat")
                    for m in range(MVT):
                        ps = p1psv.tile([128, D], dt.float32, tag="psv")
                        for nb in range(2):
                            nc.tensor.matmul(
                                ps[:, nb * 512:(nb + 1) * 512], ones128[:],
                                bv_sb[:, nb * 512:(nb + 1) * 512],
                                start=True, stop=False)
                        for t in range(KC):
                            for nb in range(2):
                                nc.tensor.matmul(
                                    ps[:, nb * 512:(nb + 1) * 512],
                                    xv_sb[:, t, m * 128:(m + 1) * 128],
                                    wv_sb[:, t, nb * 512:(nb + 1) * 512],
                                    start=False, stop=(t == KC - 1))
                        nc.vector.tensor_scalar_mul(
                            v_nat[:, m], ps[:], mv_sb[:, m:m + 1])
                    for m in range(MVT):
                        nc.gpsimd.dma_start(
                            a2v_in.rearrange("d (m p) f -> p m d f",
                                             m=MVT)[:, m],
                            v_nat[:, m].rearrange("p (d f) -> p d f",
                                                  d=N_CORES))
                    if no_collective:
                        nc.sync.dma_start(a2v_out[:], a2v_in[:])
                    else:
                        nc.gpsimd.collective_compute(
                            "AllToAll", mybir.AluOpType.bypass,
                            replica_groups=rg,
                            ins=[a2v_in.opt()], outs=[a2v_out.opt()])

                    # --- K projection (head-sharded: all kv tokens) ---
                    xk_sb = p1x.tile([128, KC, TKV], bf16, tag="xk")
                    wk_sb = p1w.tile([128, KC, 128], bf16, tag="wk")
                    nc.scalar.dma_start(
                        wk_sb[:], wk[:].rearrange("(t p) n -> p t n", p=128))
                    for t in range(KC):
                        qdma[t % 4].dma_start(
                            xk_sb[:, t], xkT[t * 128:(t + 1) * 128, :])
                    kT_sb = p1o.tile([128, TKV], bf16, tag="kT")
                    for base in range(0, TKV, 2048):
                        ng = min(4, (TKV - base) // 512)
                        kps = p1ps.tile([128, 4, 512], dt.float32, tag="ps")
                        for t in range(KC):
                            for g in range(ng):
                                col = base + g * 512
                                nc.tensor.matmul(
                                    kps[:, g], wk_sb[:, t, :],
                                    xk_sb[:, t, col:col + 512],
                                    start=(t == 0), stop=(t == KC - 1))
                        for g in range(ng):
                            col = base + g * 512
                            nc.vector.tensor_scalar_add(
                                kT_sb[:, col:col + 512], kps[:, g], bk_sb[:])

                    # --- Q projection (head-sharded: all query tokens) ---
                    xq_sb = p1x.tile([128, KC, R], bf16, tag="xq")
                    wq_sb = p1w.tile([128, KC, 128], bf16, tag="wq")
                    nc.scalar.dma_start(
                        wq_sb[:], wq[:].rearrange("(t p) n -> p t n", p=128))
                    for t in range(KC):
                        qdma[t % 4].dma_start(
                            xq_sb[:, t], xqT[t * 128:(t + 1) * 128, :])
                    qT_sb = p1o.tile([128, R], bf16, tag="qT")
                    for half in range(2):
                        qps = p1ps.tile([128, 4, 512], dt.float32, tag="ps")
                        for t in range(KC):
                            for g in range(4):
                                col = half * 2048 + g * 512
                                nc.tensor.matmul(
                                    qps[:, g], wq_sb[:, t, :],
                                    xq_sb[:, t, col:col + 512],
                                    start=(t == 0), stop=(t == KC - 1))
                        for g in range(4):
                            col = half * 2048 + g * 512
                            nc.vector.tensor_scalar_add(
                                qT_sb[:, col:col + 512], qps[:, g], bq_sb[:])

                    # --- out-proj weights (early DMA, used in phase 3) ---
                    wo_sb = p1o.tile([128, KC, D], bf16, tag="wo")
                    for t in range(KC):
                        qdma[t % 4].dma_start(
                            wo_sb[:, t], wo[t * 128:(t + 1) * 128, :])

                    # --- v_aug assembly: [128, KT, 130], per head 65 cols:
                    #     0..63 = V, col 64 = keep (denominator) ---
                    v_aug = p1o.tile([128, KT, 130], bf16, tag="vaug")
                    for h in range(HPC):
                        nc.sync.dma_start(
                            v_aug[:, :, h * 65 + 64:h * 65 + 65]
                            .rearrange("p t one -> p (t one)"),
                            keepin[:])
                    for j in range(N_CORES):
                        for h in range(HPC):
                            nc.sync.dma_start(
                                v_aug[:, j * MKT:(j + 1) * MKT,
                                      h * 65:h * 65 + 64],
                                a2v_out[j].rearrange(
                                    "(m p) (hh f) -> p m hh f",
                                    p=128, hh=HPC)[:, :, h])

                    p1ps.release()
                    p1psv.release()
                    p1w.release()
                    p1x.release()

                    if variant == "p1":
                        zt = p1o.tile([128, RPC], dt.float32, tag="zt")
                        nc.vector.memset(zt[:], 0.0)
                        for n in range(KC):
                            nc.sync.dma_start(
                                outT[n * 128:(n + 1) * 128, :], zt[:])
                        continue

                    # ============ phase 2: attention (head-sharded) =======
                    # 256-wide query blocks, swept even halves then odd
                    # halves of each destination 512-row block, so each
                    # half's AllToAll (and half of phase 3) overlaps the
                    # other half's attention compute.
                    with (
                        tc.tile_pool(name="p2p", bufs=8) as p2p,
                        tc.tile_pool(name="p2o", bufs=1) as p2o,
                        tc.tile_pool(name="p2m", bufs=2) as p2m,
                        tc.tile_pool(name="psS", bufs=3, space="PSUM") as psS,
                        tc.tile_pool(name="psO", bufs=2, space="PSUM") as psO,
                    ):
                        oT_sb = p2o.tile([128, R], bf16, tag="oT")
                        CH = 4
                        for s in range(2):
                            for c8 in range(N_CORES):
                                b = c8 // 4
                                ktb = kt0 if b == 0 else kt1
                                koff = 0 if b == 0 else kt0
                                qcol = c8 * 512 + s * 256
                                po = [psO.tile([65, 512], dt.float32, tag="o",
                                               name=f"po_h{h}")
                                      for h in range(HPC)]
                                # pss bank h holds a kt-PAIR for head h, so
                                # every PSUM bank sees a single tile_position
                                # (accumulating matmuls at a row offset with
                                # start=False hang real HW).
                                for c0 in range(0, ktb, CH):
                                    cw = min(CH, ktb - c0)
                                    p_chunk = p2p.tile([128, CH // 2, HPC,
                                                        2, 256], bf16,
                                                       tag="pch")
                                    for j0 in range(0, cw, 2):
                                        jw = min(2, cw - j0)
                                        pss = psS.tile([128, HPC, 512],
                                                       dt.float32, tag="s")
                                        for jj in range(jw):
                                            kt = koff + c0 + j0 + jj
                                            for h in range(HPC):
                                                nc.tensor.matmul(
                                                    pss[:, h,
                                                        jj * 256:
                                                        (jj + 1) * 256],
                                                    kT_sb[h * 64:(h + 1) * 64,
                                                          kt * 128:
                                                          (kt + 1) * 128],
                                                    qT_sb[h * 64:(h + 1) * 64,
                                                          qcol:qcol + 256],
                                                    start=(jj == 0),
                                                    stop=(jj == jw - 1),
                                                    tile_position=(h * 64, 0))
                                        nc.scalar.activation(
                                            p_chunk[:, j0 // 2]
                                            .rearrange("p h two x -> p (h two x)"),
                                            pss[:].rearrange(
                                                "p h x -> p (h x)"),
                                            AF.Exp, scale=float(ESC))
                                    for kk in range(cw):
                                        kt = c0 + kk
                                        for h in range(HPC):
                                            nc.tensor.matmul(
                                                po[h][:, 0:256],
                                                v_aug[:, koff + kt,
                                                      h * 65:(h + 1) * 65],
                                                p_chunk[:, kk // 2, h, kk % 2],
                                                start=(kt == 0),
                                                stop=(kt == ktb - 1))
                                for h in range(HPC):
                                    rec = p2m.tile([1, 256], f32r, tag="rec")
                                    with nc.allow_low_precision(
                                            reason="1/den at fp22 is plenty"):
                                        nc.vector.reciprocal(
                                            rec[:], po[h][64:65, 0:256])
                                    pb = psS.tile([128, HPC, 512], dt.float32,
                                                  tag="s")
                                    nc.tensor.matmul(pb[0:64, 0, 0:256],
                                                     ones64[:], rec[:],
                                                     start=True, stop=True)
                                    bc = p2p.tile([64, 256], dt.float32,
                                                  tag="bc")
                                    nc.vector.tensor_copy(bc[:],
                                                          pb[0:64, 0, 0:256])
                                    nc.vector.tensor_mul(
                                        oT_sb[h * 64:(h + 1) * 64,
                                              qcol:qcol + 256],
                                        po[h][0:64, 0:256], bc[:])

                            nc.gpsimd.dma_start(
                                a2_in[s].rearrange("d p r -> p d r"),
                                oT_sb[:].rearrange("p (d x) -> p d x",
                                                   d=N_CORES)
                                [:, :, s * 256:(s + 1) * 256])
                            if no_collective:
                                nc.sync.dma_start(a2_out[s][:], a2_in[s][:])
                            else:
                                nc.gpsimd.collective_compute(
                                    "AllToAll", mybir.AluOpType.bypass,
                                    replica_groups=rg,
                                    ins=[a2_in[s].opt()],
                                    outs=[a2_out[s].opt()])

                    if variant == "p12":
                        zt = p1o.tile([128, RPC], dt.float32, tag="zt")
                        nc.vector.memset(zt[:], 0.0)
                        for n in range(KC):
                            nc.sync.dma_start(
                                outT[n * 128:(n + 1) * 128, :], zt[:])
                        continue

                    # ============ phase 3: out projection (row-sharded) ===
                    with (
                        tc.tile_pool(name="p3a", bufs=1) as p3a,
                        tc.tile_pool(name="p3y", bufs=2) as p3y,
                        tc.tile_pool(name="p3ps", bufs=2, space="PSUM") as p3ps,
                    ):
                        aT_sb = p3a.tile([128, KC, RPC], bf16, tag="aT")
                        for s in range(2):
                            nc.sync.dma_start(
                                aT_sb[:].rearrange(
                                    "p t (two x) -> p t two x",
                                    two=2)[:, :, s],
                                a2_out[s][:].rearrange("j p r -> p j r"))
                        for n in range(KC):
                            ops = p3ps.tile([128, 512], dt.float32, tag="ps")
                            for t in range(KC):
                                nc.tensor.matmul(
                                    ops[:],
                                    wo_sb[:, t, n * 128:(n + 1) * 128],
                                    aT_sb[:, t],
                                    start=(t == 0), stop=(t == KC - 1))
                            yT = p3y.tile([128, 512], dt.float32, tag="y")
                            nc.vector.tensor_scalar_add(
                                yT[:], ops[:], bo_sb[:, n:n + 1])
                            nc.sync.dma_start(
                                outT[n * 128:(n + 1) * 128, :], yT[:])

    nc.compile()
    return nc


def _plan(mask):
    """Compaction plan from the host-visible mask (True = masked)."""
    m = np.asarray(mask).reshape(B, S)
    keep = ~m
    idx = [np.nonzero(keep[b])[0] for b in range(B)]
    kt = [max(1, -(-len(ix) // 128)) for ix in idx]
    total = kt[0] + kt[1]
    pad = (-total) % 8                        # total divisible by 8
    kt[1] += pad
    return idx, kt[0], kt[1]


def _prep(query, key, value, mask, Wq, bq, Wk, bk, Wv, bv, Wo, bo):
    f = lambda a: np.ascontiguousarray(np.asarray(a, dtype=np.float32))
    bf16np = mybir.dt.np(dt.bfloat16)
    tob = lambda a: np.ascontiguousarray(a).astype(bf16np)

    idx, kt0, kt1 = _plan(mask)
    KT = kt0 + kt1
    TKV = 128 * KT
    KVL = TKV // N_CORES
    MVT = KVL // 128

    xq = f(query).reshape(R, D)
    xk_full = f(key).reshape(B, S, D)
    xv_full = f(value).reshape(B, S, D)

    xk_c = np.zeros((TKV, D), np.float32)
    xv_c = np.zeros((TKV, D), np.float32)
    keep01 = np.zeros(TKV, np.float32)
    offs = [0, 128 * kt0]
    for b in range(B):
        n = len(idx[b])
        xk_c[offs[b]:offs[b] + n] = xk_full[b][idx[b]]
        xv_c[offs[b]:offs[b] + n] = xv_full[b][idx[b]]
        keep01[offs[b]:offs[b] + n] = 1.0

    keepin = tob(keep01.reshape(KT, 128).T)

    shared = {
        "xqT": tob(xq.T), "xkT": tob(xk_c.T), "wv": tob(f(Wv)),
        "wo": tob(f(Wo)), "bv": f(bv), "bo": f(bo), "keepin": keepin,
        "onesin": np.ones((128, 128), np.float32),
    }
    in_maps = []
    for c in range(N_CORES):
        cols = slice(c * 128, (c + 1) * 128)
        rows = slice(c * KVL, (c + 1) * KVL)
        in_maps.append({
            "xvT": tob(xv_c[rows].T),
            "wq": tob(f(Wq)[:, cols]),
            "wk": tob(f(Wk)[:, cols]),
            "bq": f(bq)[cols],
            "bk": f(bk)[cols],
            "mv01": np.ascontiguousarray(
                keep01[rows].reshape(MVT, 128).T.astype(np.float32)),
            **shared,
        })
    return in_maps


def _assemble(res):
    out = np.empty((R, D), np.float32)
    for c in range(N_CORES):
        out[c * RPC:(c + 1) * RPC] = res[c]["outT"].T
    return out.reshape(B, S, D)


def kernel(query, key, value, mask, Wq, bq, Wk, bk, Wv, bv, Wo, bo):
    _, kt0, kt1 = _plan(mask)
    bkey = (kt0, kt1)
    if bkey not in _CACHE:
        _CACHE[bkey] = _build(kt0, kt1)
    nc = _CACHE[bkey]
    in_maps = _prep(query, key, value, mask, Wq, bq, Wk, bk, Wv, bv, Wo, bo)
    res = run_bass_kernel_spmd(nc, in_maps, list(range(N_CORES)))
    return _assemble({c: res.results[c] for c in range(N_CORES)})


# revision 3
# speedup vs baseline: 1.3482x; 1.3482x over previous
"""Multi-head attention on 8 trn2 NeuronCores — v2.

Strategy vs the v1 kernel:
- Masked-key compaction: the mask is per-key (shape [B,1,1,S]) and known on
  the host, so masked keys are dropped before the K/V projections and
  attention.  ~51% of keys are masked for this input distribution, halving
  score/attn/exp/K-V-projection work.  Padding keys (to a 128 multiple)
  carry keep=0 and zeroed V so they drop out of both softmax numerator and
  denominator; no masking is needed anywhere on device.
- Head-sharded (tensor-parallel) Q/K projections: every core projects ALL
  tokens into its own 128 features (2 heads), which lands directly in the
  head-sharded attention layout — no collective between projection and
  attention.  Cost is identical to row-sharding (same output elements), only
  input DMA is bigger (bf16 host-converted).
- V is projected row-sharded (natural token-major layout falls out of the
  matmul) and redistributed with one AllToAll that overlaps the K/Q
  projections and early score tiles.
- All matmuls bf16 (fp8 was measured to cost 3e-2..6e-2 max-rel error —
  over the 2e-2 budget; bf16 end-to-end sits at ~4e-3).  Projections use
  full-PSUM weight-stationary ordering (one 2KB bank per accumulation
  group).  exp folds the 1/sqrt(dk) descale into ScalarE's activation
  scale; softmax denominator comes from a keep-flag column appended to V.
- One AllToAll moves attention output to row sharding for the out
  projection.
"""
import numpy as np

from concourse import bacc, tile, mybir
from concourse.bass_utils import run_bass_kernel_spmd

N_CORES = 8
B, S, D, H = 2, 2048, 1024, 16
DK = D // H                  # 64
R = B * S                    # 4096 query rows
RPC = R // N_CORES           # 512 rows/core for out-projection
HPC = H // N_CORES           # 2 heads/core
KC = D // 128                # 8 contraction chunks
ESC = 1.0 / np.sqrt(DK)      # exp() input descale

dt = mybir.dt
AF = mybir.ActivationFunctionType

_CACHE = {}
OUT_NAMES = ["outT"]


def _build(kt0=8, kt1=8, reps=1, no_collective=False, variant="", PEXP=2):
    KT = kt0 + kt1               # key tiles total; divisible by 8 so V
    assert KT % 8 == 0           # token rows shard evenly across cores
    TKV = 128 * KT               # compacted+padded kv tokens
    KVL = TKV // N_CORES         # local kv token rows (V projection)
    MVT = KVL // 128             # local V row tiles
    MKT = KT // N_CORES          # key tiles delivered per source core

    nc = bacc.Bacc("TRN2", target_bir_lowering=False, debug=False,
                   num_devices=N_CORES)

    f32r = dt.float32r
    bf16 = dt.bfloat16
    rg = [list(range(N_CORES))]

    # ---- per-core DRAM I/O ----
    xqT = nc.dram_tensor("xqT", [D, R], bf16, kind="ExternalInput")
    xkT = nc.dram_tensor("xkT", [D, TKV], bf16, kind="ExternalInput")
    xvT = nc.dram_tensor("xvT", [D, KVL], bf16, kind="ExternalInput")
    wq = nc.dram_tensor("wq", [D, 128], bf16, kind="ExternalInput")
    wk = nc.dram_tensor("wk", [D, 128], bf16, kind="ExternalInput")
    wv = nc.dram_tensor("wv", [D, D], bf16, kind="ExternalInput")
    wo = nc.dram_tensor("wo", [D, D], bf16, kind="ExternalInput")
    bq = nc.dram_tensor("bq", [128], dt.float32, kind="ExternalInput")
    bk = nc.dram_tensor("bk", [128], dt.float32, kind="ExternalInput")
    bv = nc.dram_tensor("bv", [D], dt.float32, kind="ExternalInput")
    bo = nc.dram_tensor("bo", [D], dt.float32, kind="ExternalInput")
    keepin = nc.dram_tensor("keepin", [128, KT], bf16, kind="ExternalInput")
    mv01 = nc.dram_tensor("mv01", [128, MVT], dt.float32,
                          kind="ExternalInput")
    onesin = nc.dram_tensor("onesin", [128, 128], dt.float32,
                            kind="ExternalInput")
    outT = nc.dram_tensor("outT", [D, RPC], dt.float32, kind="ExternalOutput")

    with tile.TileContext(nc) as tc:
        with tc.tile_pool(name="dram", bufs=1, space="DRAM") as dram:
            for rep in range(reps):
                a2v_in = dram.tile([N_CORES, KVL, 128], bf16)
                a2v_out = dram.tile([N_CORES, KVL, 128], bf16)
                a2_in = [dram.tile([N_CORES, 128, RPC // 2], bf16,
                                   name=f"a2in{s}") for s in range(2)]
                a2_out = [dram.tile([N_CORES, 128, RPC // 2], bf16,
                                    name=f"a2out{s}") for s in range(2)]
                qdma = [nc.sync, nc.scalar, nc.gpsimd, nc.sync]

                # ============ phase 1: projections (no collectives) =======
                with (
                    tc.tile_pool(name="p1o", bufs=1) as p1o,
                    tc.tile_pool(name="p1b", bufs=1) as p1b,
                ):
                    p1x = tc.alloc_tile_pool(name="p1x", bufs=1)
                    p1w = tc.alloc_tile_pool(name="p1w", bufs=1)
                    p1psv = tc.alloc_tile_pool(name="p1psv", bufs=2,
                                               space="PSUM")
                    p1ps = tc.alloc_tile_pool(name="p1ps", bufs=1,
                                              space="PSUM")

                    # --- bias / const loads (tiny) ---
                    bq_sb = p1b.tile([128, 1], dt.float32, tag="bq")
                    bk_sb = p1b.tile([128, 1], dt.float32, tag="bk")
                    mv_sb = p1b.tile([128, MVT], dt.float32, tag="mv")
                    if rep == 0:
                        nc.scalar.dma_start(
                            bq_sb[:], bq[:].rearrange("(p one) -> p one", one=1))
                        nc.scalar.dma_start(
                            bk_sb[:], bk[:].rearrange("(p one) -> p one", one=1))
                        nc.scalar.dma_start(mv_sb[:], mv01[:])
                    else:
                        # serialize reps for the x1/x5 timing differencing:
                        # garbage-but-finite values sourced from rep-1's output
                        nc.scalar.dma_start(bq_sb[:], outT[0:128, 0:1])
                        nc.scalar.dma_start(bk_sb[:], outT[0:128, 1:2])
                        nc.scalar.dma_start(mv_sb[:], outT[0:128, 2:2 + MVT])
                    bv_sb = p1b.tile([1, D], f32r, tag="bv")
                    nc.scalar.dma_start(
                        bv_sb[:],
                        bv[:].rearrange("(one f) -> one f", one=1).bitcast(f32r))
                    ones128 = p1b.tile([1, 128], f32r, tag="ones128")
                    nc.scalar.dma_start(ones128[:], onesin[0:1, :].bitcast(f32r))
                    ones64 = p1b.tile([1, 64], f32r, tag="ones64")
                    nc.scalar.dma_start(ones64[:], onesin[0:1, 0:64].bitcast(f32r))
                    bo_sb = p1b.tile([128, KC], dt.float32, tag="bo")
                    nc.scalar.dma_start(
                        bo_sb[:], bo[:].rearrange("(n p) -> p n", p=128))

                    # --- V projection (row-sharded, natural layout) ---
                    xv_sb = p1x.tile([128, KC, KVL], bf16, tag="xv")
                    wv_sb = p1w.tile([128, KC, D], bf16, tag="wv")
                    for t in range(KC):
                        qdma[t % 2].dma_start(
                            xv_sb[:, t], xvT[t * 128:(t + 1) * 128, :])
                        qdma[2 + t % 2].dma_start(
                            wv_sb[:, t], wv[t * 128:(t + 1) * 128, :])
                    v_nat = p1o.tile([128, MVT, D], bf16, tag="vnat")
                    for m in range(MVT):
                        ps = p1psv.tile([128, D], dt.float32, tag="psv")
                        for nb in range(2):
                            nc.tensor.matmul(
                                ps[:, nb * 512:(nb + 1) * 512], ones128[:],
                                bv_sb[:, nb * 512:(nb + 1) * 512],
                                start=True, stop=False)
                        for t in range(KC):
                            for nb in range(2):
                                nc.tensor.matmul(
                                    ps[:, nb * 512:(nb + 1) * 512],
                                    xv_sb[:, t, m * 128:(m + 1) * 128],
                                    wv_sb[:, t, nb * 512:(nb + 1) * 512],
                                    start=False, stop=(t == KC - 1))
                        nc.vector.tensor_scalar_mul(
                            v_nat[:, m], ps[:], mv_sb[:, m:m + 1])
                    for m in range(MVT):
                        nc.gpsimd.dma_start(
                            a2v_in.rearrange("d (m p) f -> p m d f",
                                             m=MVT)[:, m],
                            v_nat[:, m].rearrange("p (d f) -> p d f",
                                                  d=N_CORES))
                    if no_collective:
                        nc.sync.dma_start(a2v_out[:], a2v_in[:])
                    else:
                        nc.gpsimd.collective_compute(
                            "AllToAll", mybir.AluOpType.bypass,
                            replica_groups=rg,
                            ins=[a2v_in.opt()], outs=[a2v_out.opt()])

                    # --- K projection (head-sharded: all kv tokens) ---
                    xk_sb = p1x.tile([128, KC, TKV], bf16, tag="xk")
                    wk_sb = p1w.tile([128, KC, 128], bf16, tag="wk")
                    nc.scalar.dma_start(
                        wk_sb[:], wk[:].rearrange("(t p) n -> p t n", p=128))
                    for t in range(KC):
                        qdma[t % 4].dma_start(
                            xk_sb[:, t], xkT[t * 128:(t + 1) * 128, :])
                    kT_sb = p1o.tile([128, TKV], bf16, tag="kT")
                    for base in range(0, TKV, 2048):
                        ng = min(4, (TKV - base) // 512)
                        kps = p1ps.tile([128, 4, 512], dt.float32, tag="ps")
                        for t in range(KC):
                            for g in range(ng):
                                col = base + g * 512
                                nc.tensor.matmul(
                                    kps[:, g], wk_sb[:, t, :],
                                    xk_sb[:, t, col:col + 512],
                                    start=(t == 0), stop=(t == KC - 1))
                        for g in range(ng):
                            col = base + g * 512
                            nc.vector.tensor_scalar_add(
                                kT_sb[:, col:col + 512], kps[:, g], bk_sb[:])

                    # --- Q projection (head-sharded: all query tokens) ---
                    xq_sb = p1x.tile([128, KC, R], bf16, tag="xq")
                    wq_sb = p1w.tile([128, KC, 128], bf16, tag="wq")
                    nc.scalar.dma_start(
                        wq_sb[:], wq[:].rearrange("(t p) n -> p t n", p=128))
                    for t in range(KC):
                        qdma[t % 4].dma_start(
                            xq_sb[:, t], xqT[t * 128:(t + 1) * 128, :])
                    qT_sb = p1o.tile([128, R], bf16, tag="qT")
                    for half in range(2):
                        qps = p1ps.tile([128, 4, 512], dt.float32, tag="ps")
                        for t in range(KC):
                            for g in range(4):
                                col = half * 2048 + g * 512
                                nc.tensor.matmul(
                                    qps[:, g], wq_sb[:, t, :],
                                    xq_sb[:, t, col:col + 512],
                                    start=(t == 0), stop=(t == KC - 1))
                        for g in range(4):
                            col = half * 2048 + g * 512
                            nc.vector.tensor_scalar_add(
                                qT_sb[:, col:col + 512], qps[:, g], bq_sb[:])

                    # --- out-proj weights (early DMA, used in phase 3) ---
                    wo_sb = p1o.tile([128, KC, D], bf16, tag="wo")
                    for t in range(KC):
                        qdma[t % 4].dma_start(
                            wo_sb[:, t], wo[t * 128:(t + 1) * 128, :])

                    # --- v_aug assembly: [128, KT, 130], per head 65 cols:
                    #     0..63 = V, col 64 = keep (denominator) ---
                    v_aug = p1o.tile([128, KT, 130], bf16, tag="vaug")
                    for h in range(HPC):
                        nc.sync.dma_start(
                            v_aug[:, :, h * 65 + 64:h * 65 + 65]
                            .rearrange("p t one -> p (t one)"),
                            keepin[:])
                    for j in range(N_CORES):
                        for h in range(HPC):
                            nc.sync.dma_start(
                                v_aug[:, j * MKT:(j + 1) * MKT,
                                      h * 65:h * 65 + 64],
                                a2v_out[j].rearrange(
                                    "(m p) (hh f) -> p m hh f",
                                    p=128, hh=HPC)[:, :, h])

                    p1ps.release()
                    p1psv.release()
                    p1w.release()
                    p1x.release()

                    if variant == "p1":
                        zt = p1o.tile([128, RPC], dt.float32, tag="zt")
                        nc.vector.memset(zt[:], 0.0)
                        for n in range(KC):
                            nc.sync.dma_start(
                                outT[n * 128:(n + 1) * 128, :], zt[:])
                        continue

                    # ============ phase 2: attention (head-sharded) =======
                    # 256-wide query blocks, swept even halves then odd
                    # halves of each destination 512-row block, so each
                    # half's AllToAll (and half of phase 3) overlaps the
                    # other half's attention compute.
                    with (
                        tc.tile_pool(name="p2p", bufs=8) as p2p,
                        tc.tile_pool(name="p2o", bufs=1) as p2o,
                        tc.tile_pool(name="p2m", bufs=2) as p2m,
                        tc.tile_pool(name="psS", bufs=3, space="PSUM") as psS,
                        tc.tile_pool(name="psO", bufs=2, space="PSUM") as psO,
                    ):
                        oT_sb = p2o.tile([128, R], bf16, tag="oT")
                        CH = 4
                        for s in range(2):
                            for c8 in range(N_CORES):
                                b = c8 // 4
                                ktb = kt0 if b == 0 else kt1
                                koff = 0 if b == 0 else kt0
                                qcol = c8 * 512 + s * 256
                                po = [psO.tile([65, 512], dt.float32, tag="o",
                                               name=f"po_h{h}")
                                      for h in range(HPC)]
                                # pss bank h holds a kt-PAIR for head h, so
                                # every PSUM bank sees a single tile_position
                                # (accumulating matmuls at a row offset with
                                # start=False hang real HW).
                                for c0 in range(0, ktb, CH):
                                    cw = min(CH, ktb - c0)
                                    p_chunk = p2p.tile([128, CH // 2, HPC,
                                                        2, 256], bf16,
                                                       tag="pch")
                                    for j0 in range(0, cw, 2):
                                        jw = min(2, cw - j0)
                                        pss = psS.tile([128, HPC, 512],
                                                       dt.float32, tag="s")
                                        for jj in range(jw):
                                            kt = koff + c0 + j0 + jj
                                            for h in range(HPC):
                                                nc.tensor.matmul(
                                                    pss[:, h,
                                                        jj * 256:
                                                        (jj + 1) * 256],
                                                    kT_sb[h * 64:(h + 1) * 64,
                                                          kt * 128:
                                                          (kt + 1) * 128],
                                                    qT_sb[h * 64:(h + 1) * 64,
                                                          qcol:qcol + 256],
                                                    start=(jj == 0),
                                                    stop=(jj == jw - 1),
                                                    tile_position=(h * 64, 0))
                                        nc.scalar.activation(
                                            p_chunk[:, j0 // 2]
                                            .rearrange("p h two x -> p (h two x)"),
                                            pss[:].rearrange(
                                                "p h x -> p (h x)"),
                                            AF.Exp, scale=float(ESC))
                                    for kk in range(cw):
                                        kt = c0 + kk
                                        for h in range(HPC):
                                            nc.tensor.matmul(
                                                po[h][:, 0:256],
                                                v_aug[:, koff + kt,
                                                      h * 65:(h + 1) * 65],
                                                p_chunk[:, kk // 2, h, kk % 2],
                                                start=(kt == 0),
                                                stop=(kt == ktb - 1))
                                for h in range(HPC):
                                    rec = p2m.tile([1, 256], f32r, tag="rec")
                                    with nc.allow_low_precision(
                                            reason="1/den at fp22 is plenty"):
                                        nc.vector.reciprocal(
                                            rec[:], po[h][64:65, 0:256])
                                    pb = psS.tile([128, HPC, 512], dt.float32,
                                                  tag="s")
                                    nc.tensor.matmul(pb[0:64, 0, 0:256],
                                                     ones64[:], rec[:],
                                                     start=True, stop=True)
                                    bc = p2p.tile([64, 256], dt.float32,
                                                  tag="bc")
                                    nc.vector.tensor_copy(bc[:],
                                                          pb[0:64, 0, 0:256])
                                    nc.vector.tensor_mul(
                                        oT_sb[h * 64:(h + 1) * 64,
                                              qcol:qcol + 256],
                                        po[h][0:64, 0:256], bc[:])

                            nc.gpsimd.dma_start(
                                a2_in[s].rearrange("d p r -> p d r"),
                                oT_sb[:].rearrange("p (d x) -> p d x",
                                                   d=N_CORES)
                                [:, :, s * 256:(s + 1) * 256])
                            if no_collective:
                                nc.sync.dma_start(a2_out[s][:], a2_in[s][:])
                            else:
                                nc.gpsimd.collective_compute(
                                    "AllToAll", mybir.AluOpType.bypass,
                                    replica_groups=rg,
                                    ins=[a2_in[s].opt()],
                                    outs=[a2_out[s].opt()])

                    if variant == "p12":
                        zt = p1o.tile([128, RPC], dt.float32, tag="zt")
                        nc.vector.memset(zt[:], 0.0)
                        for n in range(KC):
                            nc.sync.dma_start(
                                outT[n * 128:(n + 1) * 128, :], zt[:])
                        continue

                    # ============ phase 3: out projection (row-sharded) ===
                    # two 256-row passes: pass 0 depends only on the first
                    # AllToAll, so its matmuls run while AllToAll #1 is
                    # still in flight.
                    with (
                        tc.tile_pool(name="p3a", bufs=2) as p3a,
                        tc.tile_pool(name="p3y", bufs=2) as p3y,
                        tc.tile_pool(name="p3ps", bufs=2, space="PSUM") as p3ps,
                    ):
                        for s in range(2):
                            aT_sb = p3a.tile([128, KC, RPC // 2], bf16,
                                             tag="aT", name=f"aT{s}")
                            nc.sync.dma_start(
                                aT_sb[:],
                                a2_out[s][:].rearrange("j p r -> p j r"))
                            for n in range(KC):
                                ops = p3ps.tile([128, 512], dt.float32,
                                                tag="ps")
                                for t in range(KC):
                                    nc.tensor.matmul(
                                        ops[:, 0:256],
                                        wo_sb[:, t, n * 128:(n + 1) * 128],
                                        aT_sb[:, t],
                                        start=(t == 0), stop=(t == KC - 1))
                                yT = p3y.tile([128, 256], dt.float32,
                                              tag="y")
                                nc.vector.tensor_scalar_add(
                                    yT[:], ops[:, 0:256], bo_sb[:, n:n + 1])
                                nc.sync.dma_start(
                                    outT[n * 128:(n + 1) * 128,
                                         s * 256:(s + 1) * 256], yT[:])

    nc.compile()
    return nc


def _plan(mask):
    """Compaction plan from the host-visible mask (True = masked)."""
    m = np.asarray(mask).reshape(B, S)
    keep = ~m
    idx = [np.nonzero(keep[b])[0] for b in range(B)]
    kt = [max(1, -(-len(ix) // 128)) for ix in idx]
    total = kt[0] + kt[1]
    pad = (-total) % 8                        # total divisible by 8
    kt[1] += pad
    return idx, kt[0], kt[1]


def _prep(query, key, value, mask, Wq, bq, Wk, bk, Wv, bv, Wo, bo):
    f = lambda a: np.ascontiguousarray(np.asarray(a, dtype=np.float32))
    bf16np = mybir.dt.np(dt.bfloat16)
    tob = lambda a: np.ascontiguousarray(a).astype(bf16np)

    idx, kt0, kt1 = _plan(mask)
    KT = kt0 + kt1
    TKV = 128 * KT
    KVL = TKV // N_CORES
    MVT = KVL // 128

    xq = f(query).reshape(R, D)
    xk_full = f(key).reshape(B, S, D)
    xv_full = f(value).reshape(B, S, D)

    xk_c = np.zeros((TKV, D), np.float32)
    xv_c = np.zeros((TKV, D), np.float32)
    keep01 = np.zeros(TKV, np.float32)
    offs = [0, 128 * kt0]
    for b in range(B):
        n = len(idx[b])
        xk_c[offs[b]:offs[b] + n] = xk_full[b][idx[b]]
        xv_c[offs[b]:offs[b] + n] = xv_full[b][idx[b]]
        keep01[offs[b]:offs[b] + n] = 1.0

    keepin = tob(keep01.reshape(KT, 128).T)

    shared = {
        "xqT": tob(xq.T), "xkT": tob(xk_c.T), "wv": tob(f(Wv)),
        "wo": tob(f(Wo)), "bv": f(bv), "bo": f(bo), "keepin": keepin,
        "onesin": np.ones((128, 128), np.float32),
    }
    in_maps = []
    for c in range(N_CORES):
        cols = slice(c * 128, (c + 1) * 128)
        rows = slice(c * KVL, (c + 1) * KVL)
        in_maps.append({
            "xvT": tob(xv_c[rows].T),
            "wq": tob(f(Wq)[:, cols]),
            "wk": tob(f(Wk)[:, cols]),
            "bq": f(bq)[cols],
            "bk": f(bk)[cols],
            "mv01": np.ascontiguousarray(
                keep01[rows].reshape(MVT, 128).T.astype(np.float32)),
            **shared,
        })
    return in_maps


def _assemble(res):
    out = np.empty((R, D), np.float32)
    for c in range(N_CORES):
        out[c * RPC:(c + 1) * RPC] = res[c]["outT"].T
    return out.reshape(B, S, D)


def kernel(query, key, value, mask, Wq, bq, Wk, bk, Wv, bv, Wo, bo):
    _, kt0, kt1 = _plan(mask)
    bkey = (kt0, kt1)
    if bkey not in _CACHE:
        _CACHE[bkey] = _build(kt0, kt1)
    nc = _CACHE[bkey]
    in_maps = _prep(query, key, value, mask, Wq, bq, Wk, bk, Wv, bv, Wo, bo)
    res = run_bass_kernel_spmd(nc, in_maps, list(range(N_CORES)))
    return _assemble({c: res.results[c] for c in range(N_CORES)})
